# revision 67
# baseline (speedup 1.0000x reference)
"""Trainium2 Bass kernel for nn_DeformableRead (deformable attention read).

8 NeuronCores SPMD. Cells (anchor positions) from BOTH batches are sorted by
token count and dealt round-robin to the 8 cores, so every core sees a nearly
identical occupancy profile (~2050 tokens). Slots are packed into 128-wide
chunks of uniform per-cell capacity (capacity classes derived from the input's
occupancy at runtime), shrinking the slot count ~2x vs a fixed-capacity
layout; the Bass module is built/compiled for the derived chunk profile.
Key move: sample points live in fixed windows around each anchor cell
(9x9/5x5/4x4 at L2/L3/L4); bilinear sampling over a window is a dense 122-tap
PE contraction with separable hat weights relu(1-|xi-i|) -- gather-free.
Hat/outer-product construction alternates whole chunks between Vector and
GpSimd; the per-point m-sum of the outer products is folded into the PE via
4 accumulating transposes per head (f32 PSUM), removing both DVE add
stages. PSUM reads (extraction/casts) stay on Vector/Scalar. Patch blobs are padded to 128 DMA partitions so transfers
spread across all 16 DMA engines (122-partition DMAs land on only 2).
Output projections are staged in SBUF and flushed every 2 chunks.
Host does layout only: sharding, slot permutation, map transpose/pad/bf16,
fourier features of raw coords, constants. Device does all heavy math.
"""

import numpy as np
import ml_dtypes

import concourse.bass as bass
import concourse.bacc as bacc
import concourse.tile as tile
from concourse import mybir
from concourse.bass_utils import run_bass_kernel_spmd

D, H, NL, M = 192, 6, 3, 4
NF = 8
SIGMAS = (4.0, 2.0, 1.0)
WXY = (9, 5, 4)
CLO = (4.0, 2.0, 1.5)
PADL = (2, 1, 1)
SCALE = (4, 2, 1)
KWIN = sum(w * w for w in WXY)  # 122
LOFF = (0, WXY[0] ** 2, WXY[0] ** 2 + WXY[1] ** 2)
HATW = sum(4 * w for w in WXY)  # 72
HOFF = (0, 36, 56)
HATB = 6 * HATW  # 432, per-coord hat block
BF16 = mybir.dt.bfloat16
F32 = mybir.dt.float32

_CACHE = {}


def _ap(base, free_off, dims):
    """Custom AP: base tile slice (sets partition range), explicit free dims."""
    return bass.AP(tensor=base.tensor, offset=base.offset + free_off,
                   ap=[base.ap[0]] + [list(d) for d in dims])


def _build_module(chunks):
    """chunks: tuple of (cap, m) per 128-slot chunk; all cells in a chunk
    share capacity cap; m cells per chunk."""
    NCH = len(chunks)
    S = NCH * 128
    PBC = [m * D for (cap, m) in chunks]     # pblob col count per chunk
    PBOFF = np.concatenate([[0], np.cumsum(PBC)]).tolist()
    PBTOT = PBOFF[-1]
    MMAX = max(m for _, m in chunks)

    nc = bacc.Bacc("TRN2", target_bir_lowering=False, debug=False)
    dt = nc.dram_tensor
    uinT = dt("uinT", [2 * D + 32, S], BF16, kind="ExternalInput")
    pblobs = [dt(f"pblob{q}", [KWIN, PBC[q]], BF16, kind="ExternalInput")
              for q in range(NCH)]
    wu = dt("wu", [2 * D + 32, D], BF16, kind="ExternalInput")
    # cvec packs all [P,1] f32 bias/scale consts as columns:
    # 0:wub0 1:wub1 2:lng0 3:lng1 4:lnb0 5:lnb1 6:bda(112) 7:bdb(32)
    # 8:ba(72) 9:siga(112) 10:sigb(32) 11:cloa(112) 12:clob(32)
    # 13:bo0 14:bo1 15:eps
    cvec = dt("cvec", [112, 16], F32, kind="ExternalInput")
    wda = dt("wda", [D, 240], BF16, kind="ExternalInput")
    bd6 = dt("bd6", [72, 72], BF16, kind="ExternalInput")
    iotah = dt("iotah", [128, 2 * HATB], F32, kind="ExternalInput")
    onesw = dt("onesw", [96, 96], BF16, kind="ExternalInput")
    identf = dt("identf", [128, 128], F32, kind="ExternalInput")
    identb = dt("identb", [128, 128], BF16, kind="ExternalInput")
    wo1 = dt("wo1", [128, D], BF16, kind="ExternalInput")
    wo2 = dt("wo2", [64, D], BF16, kind="ExternalInput")
    outT = dt("outT", [D, S], F32, kind="ExternalOutput")

    NCS = [(i * 512, min(512, S - i * 512)) for i in range((S + 511) // 512)]
    AF = mybir.ActivationFunctionType
    OP = mybir.AluOpType

    with tile.TileContext(nc) as tc:
        with (
            tc.tile_pool(name="const", bufs=1) as cpool,
            tc.tile_pool(name="big", bufs=1) as bpool,
        ):
            _sbn = [0]
            def sb(t_ap, shape, dtype):
                _sbn[0] += 1
                nm = f"cst{_sbn[0]}"
                x = cpool.tile(shape, dtype, tag=nm, name=nm)
                nc.gpsimd.dma_start(x[:], t_ap)
                return x

            s_wu = []
            for kc in range(4):
                k0, k1 = kc * 128, min((kc + 1) * 128, 416)
                s_wu.append(sb(wu[k0:k1, :], [k1 - k0, D], BF16))
            s_wda = [sb(wda[0:96, :], [96, 240], BF16),
                     sb(wda[96:192, :], [96, 240], BF16)]
            s_cv = sb(cvec[:], [112, 16], F32)
            s_wub = [s_cv[0:96, 0:1], s_cv[0:96, 1:2]]
            s_lng = [s_cv[0:96, 2:3], s_cv[0:96, 3:4]]
            s_lnb = [s_cv[0:96, 4:5], s_cv[0:96, 5:6]]
            s_bda = s_cv[0:112, 6:7]
            s_bdb = s_cv[0:32, 7:8]
            s_ba = s_cv[0:72, 8:9]
            s_siga = s_cv[0:112, 9:10]
            s_sigb = s_cv[0:32, 10:11]
            s_cloa = s_cv[0:112, 11:12]
            s_clob = s_cv[0:32, 12:13]
            s_bo = [s_cv[0:96, 13:14], s_cv[0:96, 14:15]]
            s_eps = s_cv[0:96, 15:16]
            s_bd6 = sb(bd6[:], [72, 72], BF16)
            s_iota = sb(iotah[:], [128, 2 * HATB], F32)
            s_ones = sb(onesw[0:96, :], [96, 96], BF16)
            s_idf = sb(identf[:], [128, 128], F32)
            s_idb = sb(identb[:], [128, 128], BF16)
            s_wo1 = sb(wo1[:], [128, D], BF16)
            s_wo2 = sb(wo2[:], [64, D], BF16)

            u_r = [bpool.tile([96, S], BF16, tag="ur0", name="ur0"),
                   bpool.tile([96, S], BF16, tag="ur1", name="ur1")]
            xa = bpool.tile([112, S], F32, tag="xa")
            xb = bpool.tile([104, S], F32, tag="xb")

            # ======== phases A-E (column-major: features x slots) ========
            with (
                tc.tile_pool(name="wk", bufs=3) as wpool,
                tc.tile_pool(name="ucp", bufs=3) as ucpool,
                tc.tile_pool(name="psC", bufs=1, space="PSUM") as psC,
                tc.tile_pool(name="psA", bufs=3, space="PSUM") as psA,
                tc.tile_pool(name="psB", bufs=1, space="PSUM") as psB,
            ):
                for n0, nn in NCS:
                    uc = ucpool.tile([128, 4, 512], BF16, tag="uc")
                    for kc in range(4):
                        k0, k1 = kc * 128, min((kc + 1) * 128, 416)
                        nc.sync.dma_start(uc[:k1 - k0, kc, :nn],
                                           uinT[k0:k1, n0:n0 + nn])
                    y = [wpool.tile([96, 512], F32, tag="ya", name="ya"),
                         wpool.tile([96, 512], F32, tag="yc", name="yc")]
                    for mc in range(2):
                        pu = psA.tile([96, 512], F32, tag="pu")
                        for kc in range(4):
                            kk = min(128, 416 - kc * 128)
                            nc.tensor.matmul(
                                pu[:, :nn],
                                s_wu[kc][:, mc * 96:(mc + 1) * 96],
                                uc[:kk, kc, :nn],
                                start=(kc == 0), stop=(kc == 3))
                        nc.scalar.activation(
                            out=y[mc][:, :nn], in_=pu[:, :nn],
                            func=AF.Gelu,
                            bias=s_wub[mc], scale=1.0)
                    y2 = [wpool.tile([96, 512], BF16, tag="y2a", name="y2a"),
                          wpool.tile([96, 512], BF16, tag="y2c", name="y2c")]
                    nc.scalar.activation(out=y2[0][:, :nn], in_=y[0][:, :nn],
                                         func=AF.Square)
                    nc.gpsimd.tensor_mul(y2[1][:, :nn], y[1][:, :nn], y[1][:, :nn])
                    yb = [wpool.tile([96, 512], BF16, tag="yba", name="yba"),
                          wpool.tile([96, 512], BF16, tag="ybc", name="ybc")]
                    nc.vector.tensor_copy(yb[0][:, :nn], y[0][:, :nn])
                    nc.gpsimd.tensor_copy(yb[1][:, :nn], y[1][:, :nn])
                    pst = psB.tile([96, 2, 512], F32, tag="pst")
                    for st, srcs in ((0, yb), (1, y2)):
                        for kc in range(2):
                            nc.tensor.matmul(
                                pst[:, st, :nn],
                                s_ones[:],
                                srcs[kc][:, :nn],
                                start=(kc == 0), stop=(kc == 1))
                    mu = wpool.tile([96, 512], F32, tag="mu")
                    nc.vector.tensor_scalar_mul(
                        out=mu[:, :nn], in0=pst[:, 0, :nn], scalar1=1.0 / D)
                    mu2 = wpool.tile([96, 512], F32, tag="mu2")
                    nc.gpsimd.tensor_mul(mu2[:, :nn], mu[:, :nn], mu[:, :nn])
                    var = wpool.tile([96, 512], F32, tag="var")
                    nc.vector.scalar_tensor_tensor(
                        out=var[:, :nn], in0=pst[:, 1, :nn], scalar=1.0 / D,
                        in1=mu2[:, :nn], op0=OP.mult, op1=OP.subtract)
                    sd = wpool.tile([96, 512], F32, tag="sd")
                    nc.scalar.activation(out=sd[:, :nn], in_=var[:, :nn],
                                         func=AF.Sqrt, bias=s_eps, scale=1.0)
                    rr = wpool.tile([96, 512], F32, tag="rr")
                    nc.vector.reciprocal_approx_fast(out=rr[:, :nn], in_=sd[:, :nn])
                    for mc in range(2):
                        ym = wpool.tile([96, 512], F32, tag="ym")
                        eng = nc.vector if mc == 0 else nc.gpsimd
                        eng.tensor_sub(ym[:, :nn], y[mc][:, :nn],
                                       mu[:, :nn])
                        eng.tensor_mul(ym[:, :nn], ym[:, :nn], rr[:, :nn])
                        eng.tensor_scalar(
                            out=u_r[mc][:, n0:n0 + nn],
                            in0=ym[:, :nn],
                            scalar1=s_lng[mc],
                            scalar2=s_lnb[mc],
                            op0=OP.mult, op1=OP.add)

                    pda = psC.tile([128, 2, 512], F32, tag="pda")
                    for mc, (w0, w1) in enumerate(((0, 112), (112, 240))):
                        for kc in range(2):
                            nc.tensor.matmul(
                                pda[:w1 - w0, mc, :nn],
                                s_wda[kc][:, w0:w1],
                                u_r[kc][:, n0:n0 + nn],
                                start=(kc == 0), stop=(kc == 1))
                    tha = wpool.tile([112, 512], F32, tag="tha")
                    nc.scalar.activation(out=tha[:, :nn], in_=pda[0:112, 0, :nn],
                                         func=AF.Tanh, bias=s_bda, scale=1.0)
                    thb = wpool.tile([32, 512], F32, tag="thb")
                    nc.scalar.activation(out=thb[:, :nn],
                                         in_=pda[96:128, 1, :nn],
                                         func=AF.Tanh, bias=s_bdb, scale=1.0)
                    nc.vector.tensor_scalar(
                        out=xa[:, n0:n0 + nn], in0=tha[:, :nn],
                        scalar1=s_siga, scalar2=s_cloa,
                        op0=OP.mult, op1=OP.add)
                    nc.gpsimd.tensor_scalar(
                        out=xb[0:32, n0:n0 + nn], in0=thb[:, :nn],
                        scalar1=s_sigb, scalar2=s_clob,
                        op0=OP.mult, op1=OP.add)
                    ex = wpool.tile([72, 512], F32, tag="ex")
                    nc.scalar.activation(out=ex[:, :nn], in_=pda[0:72, 1, :nn],
                                         func=AF.Exp, bias=s_ba, scale=1.0)
                    exb = wpool.tile([72, 512], BF16, tag="exb")
                    nc.scalar.copy(out=exb[:, :nn], in_=ex[:, :nn])
                    pz = psB.tile([72, 512], F32, tag="pz")
                    nc.tensor.matmul(pz[:, :nn], s_bd6[:], exb[:, :nn],
                                     start=True, stop=True)
                    rz = wpool.tile([72, 512], F32, tag="rz")
                    nc.vector.reciprocal_approx_fast(out=rz[:, :nn], in_=pz[:, :nn])
                    for gi, (a0, a1) in enumerate(((0, 32), (32, 64), (64, 72))):
                        eng = nc.gpsimd if gi == 1 else nc.vector
                        eng.tensor_mul(
                            xb[32 + a0:32 + a1, n0:n0 + nn],
                            ex[a0:a1, :nn], rz[a0:a1, :nn])

            # ======== phases F-I per 128-slot chunk ========
            with (
                tc.tile_pool(name="kw", bufs=4) as kpool,
                tc.tile_pool(name="pp", bufs=4) as ppool,
                tc.tile_pool(name="odp", bufs=3) as opool,
                tc.tile_pool(name="psT", bufs=1, space="PSUM") as psT,
                tc.tile_pool(name="psK", bufs=1, space="PSUM") as psK,
                tc.tile_pool(name="psX", bufs=1, space="PSUM") as psX,
                tc.tile_pool(name="psD", bufs=2, space="PSUM") as psD,
            ):
                odg = None
                for q, (cap, m) in enumerate(chunks):
                    c0 = q * 128
                    dve = nc.gpsimd if q % 2 == 1 else nc.vector
                    scols = m * cap
                    npair, modd = m // 2, m % 2
                    rm = kpool.tile([128, 216], F32, tag="rm")
                    for (srct, dcol, wid) in ((xa, 0, 112), (xb, 112, 104)):
                        pT = psT.tile([128, 112], F32, tag="pT")
                        nc.tensor.transpose(pT[:, :wid],
                                            srct[:, c0:c0 + 128],
                                            s_idf[:wid, :wid])
                        nc.scalar.copy(out=rm[:, dcol:dcol + wid],
                                       in_=pT[:, :wid])

                    # hats hxy [128, 864] bf16: x block then y block; per-h stride 72
                    hxy = kpool.tile([128, 2 * HATB], BF16, tag="hxy")
                    for coord in range(2):
                        for l in range(NL):
                            w = WXY[l]
                            out_ap = _ap(hxy[:], coord * HATB + HOFF[l],
                                         [[72, 6], [w, 4], [1, w]])
                            in0 = _ap(rm[:], 8 * l + coord,
                                      [[24, 6], [2, 4], [0, w]])
                            in1 = _ap(s_iota[:], coord * HATB + HOFF[l],
                                      [[72, 6], [w, 4], [1, w]])
                            dve.tensor_sub(out_ap, in0, in1)
                    nc.scalar.activation(out=hxy[:], in_=hxy[:], func=AF.Abs)
                    nc.scalar.activation(out=hxy[:], in_=hxy[:], func=AF.Relu,
                                         bias=1.0, scale=-1.0)
                    for l in range(NL):
                        w = WXY[l]
                        hy_ap = _ap(hxy[:], HATB + HOFF[l],
                                    [[72, 6], [w, 4], [1, w]])
                        wt_ap = _ap(rm[:], 144 + 4 * l,
                                    [[12, 6], [1, 4], [0, w]])
                        dve.tensor_mul(hy_ap, hy_ap, wt_ap)

                    # outer products into tmp [128, (h, m, tap)] bf16;
                    # the m-sum happens on the PE via 4 accumulating
                    # transposes per head
                    tmp = kpool.tile([128, 24 * KWIN], F32, tag="tmp")
                    for l in range(NL):
                        w = WXY[l]
                        for mm in range(4):
                            hy = _ap(hxy[:], HATB + HOFF[l] + mm * w,
                                     [[72, 6], [1, w], [0, w]])
                            hx = _ap(hxy[:], HOFF[l] + mm * w,
                                     [[72, 6], [0, w], [1, w]])
                            t1 = _ap(tmp[:], mm * KWIN + LOFF[l],
                                     [[4 * KWIN, 6], [w, w], [1, w]])
                            dve.tensor_mul(t1, hy, hx)

                    # transpose-accumulate kappa per h -> pK [122, 6, 128]
                    pK = psK.tile([KWIN, 6, 128], F32, tag="pK")
                    for hh in range(H):
                        for mm in range(4):
                            nc.tensor.matmul(
                                pK[:, hh, :],
                                tmp[:, (hh * 4 + mm) * KWIN:
                                    (hh * 4 + mm + 1) * KWIN],
                                s_idf[:], is_transpose=True,
                                start=(mm == 0), stop=(mm == 3))
                    kT = kpool.tile([KWIN, 6, 128], BF16, tag="kT")
                    nc.vector.tensor_copy(kT[:], pK[:])
                    kT = kpool.tile([KWIN, 6, 128], BF16, tag="kT")
                    nc.vector.tensor_copy(kT[:, 0:4, :], pK[:, 0:4, :])
                    nc.scalar.copy(out=kT[:, 4:6, :], in_=pK[:, 4:6, :])

                    # sampling matmuls: patch cols = [m*128 ch-lo | m*64 ch-hi]
                    # DMA split over several queues (sync/scalar/tensor) so the
                    # 12MB blob spreads across DMA engines
                    patch = ppool.tile([KWIN, MMAX * D], BF16, tag="patch")
                    nc.sync.dma_start(patch[:, :m * 128],
                                      pblobs[q][:, :m * 128])
                    nc.scalar.dma_start(patch[:, m * 128:m * D],
                                        pblobs[q][:, m * 128:m * D])
                    pX = psX.tile([128, 768], F32, tag="pX")
                    pXh = psX.tile([128, 256], F32, tag="pXh")
                    for j in range(m):
                        rh = _ap(kT[:], j * cap, [[128, 6], [1, cap]])
                        nc.tensor.matmul(pX[:, j * 6 * cap:(j + 1) * 6 * cap],
                                         patch[:KWIN, j * 128:(j + 1) * 128],
                                         rh,
                                         start=True, stop=True)
                    hibase = m * 128
                    for p in range(npair):
                        rh2 = _ap(kT[:], 4 * 128 + 2 * p * cap,
                                  [[128, 2], [1, 2 * cap]])
                        nc.tensor.matmul(
                            pXh[:, p * 4 * cap:(p + 1) * 4 * cap],
                            patch[:KWIN, hibase + p * 128:hibase + (p + 1) * 128],
                            rh2, start=True, stop=True)
                    if modd:
                        rh2 = _ap(kT[:], 4 * 128 + (m - 1) * cap,
                                  [[128, 2], [1, cap]])
                        nc.tensor.matmul(
                            pXh[:64, npair * 4 * cap:
                                npair * 4 * cap + 2 * cap],
                            patch[:KWIN,
                                  hibase + npair * 128:hibase + npair * 128 + 64],
                            rh2, start=True, stop=True)

                    XU = kpool.tile([128, 128], BF16, tag="XU")
                    XL = kpool.tile([64, 128], BF16, tag="XL")
                    for hh in range(4):
                        eng = [nc.vector, nc.scalar, nc.scalar, nc.scalar][hh]
                        if eng is nc.scalar:
                            eng.copy(
                                out=_ap(XU[32 * hh:32 * hh + 32, :], 0,
                                        [[cap, m], [1, cap]]),
                                in_=_ap(pX[32 * hh:32 * hh + 32, :], hh * cap,
                                        [[6 * cap, m], [1, cap]]))
                        else:
                            eng.tensor_copy(
                                _ap(XU[32 * hh:32 * hh + 32, :], 0,
                                    [[cap, m], [1, cap]]),
                                _ap(pX[32 * hh:32 * hh + 32, :], hh * cap,
                                    [[6 * cap, m], [1, cap]]))
                    # XL: rows 0:64 of psum hold even cells' ch-hi, 64:128 odd
                    for hp in range(2):
                        for par in range(2):
                            if npair == 0:
                                continue
                            src = _ap(pXh[64 * par + 32 * hp:
                                          64 * par + 32 * hp + 32, :],
                                      hp * 2 * cap + par * cap,
                                      [[4 * cap, npair], [1, cap]])
                            dst = _ap(XL[32 * hp:32 * hp + 32, :], par * cap,
                                      [[2 * cap, npair], [1, cap]])
                            if hp == 0 and par == 0:
                                nc.vector.tensor_copy(dst, src)
                            else:
                                nc.scalar.copy(out=dst, in_=src)
                    if modd:
                        for hp in range(2):
                            nc.scalar.copy(
                                out=_ap(XL[32 * hp:32 * hp + 32, :],
                                        (m - 1) * cap, [[1, cap]]),
                                in_=_ap(pXh[32 * hp:32 * hp + 32, :],
                                        npair * 4 * cap + hp * cap,
                                        [[1, cap]]))

                    # output proj, staged in SBUF and flushed every 4 chunks
                    qg = q % 2
                    if qg == 0:
                        odg = opool.tile([96, 2, 256], F32, tag="odg")
                    for mc in range(2):
                        pD = psD.tile([96, 128], F32, tag="pD")
                        nc.tensor.matmul(pD[:, :scols],
                                         s_wo1[:, mc * 96:(mc + 1) * 96],
                                         XU[:, :scols], start=True, stop=False)
                        nc.tensor.matmul(pD[:, :scols],
                                         s_wo2[:, mc * 96:(mc + 1) * 96],
                                         XL[:, :scols], start=False, stop=True)
                        if mc == 0:
                            nc.vector.tensor_scalar_add(
                                out=odg[:, mc, qg * 128:qg * 128 + scols],
                                in0=pD[:, :scols], scalar1=s_bo[mc])
                        else:
                            nc.scalar.add(odg[:, mc, qg * 128:qg * 128 + scols],
                                          pD[:, :scols], s_bo[mc])
                    if qg == 1 or q == NCH - 1:
                        g0 = (q - qg) * 128
                        gw = qg * 128 + 128
                        for mc in range(2):
                            nc.sync.dma_start(
                                outT[mc * 96:(mc + 1) * 96, g0:g0 + gw],
                                odg[:, mc, :gw])
    nc.compile()
    return nc


def _plan(ti):
    """Cell dealing + capacity chunking from actual occupancy."""
    B = ti.shape[0]
    counts = np.zeros((B, 1024), np.int64)
    for b in range(B):
        np.add.at(counts[b], ti[b].ravel(), 1)
    allc = counts.ravel()
    order = np.argsort(-allc, kind='stable')
    capr = allc[order[0::8]]  # per-rank capacity (max across cores)
    chunks = []
    r = 0
    while r < 256:
        cap = min(max(int(capr[r]), 1), 128)
        m = min(128 // cap, 256 - r)
        chunks.append((cap, m))
        r += m
    # core i, rank r -> global cell order[8*r + i]
    deal = order.reshape(256, 8)
    return tuple(chunks), deal, allc


def _host_prep(inputs, chunks, deal, allc):
    h = inputs["h"].astype(np.float32)
    ti = inputs["top_indices"].astype(np.int64)
    qc = inputs["query_coords"].astype(np.float32)
    g = inputs["g"].astype(np.float32)
    maps = [np.asarray(inputs["L2_proj"], np.float32),
            np.asarray(inputs["L3_proj"], np.float32),
            np.asarray(inputs["L4_proj"], np.float32)]
    B, K, R = ti.shape
    NCH = len(chunks)
    S = NCH * 128

    consts = {}
    consts["wu"] = np.ascontiguousarray(inputs["w_u_w"].T).astype(ml_dtypes.bfloat16)
    wda = np.concatenate([inputs["w_delta_w"][0:112], inputs["w_a_w"],
                          np.zeros((24, D), np.float32),
                          inputs["w_delta_w"][112:144]], 0)
    consts["wda"] = np.ascontiguousarray(wda.T).astype(ml_dtypes.bfloat16)
    sg = np.zeros((H, NL, M, 2), np.float32)
    cloa = np.zeros((H, NL, M, 2), np.float32)
    for l in range(NL):
        sg[:, l] = SIGMAS[l]
        cloa[:, l] = CLO[l]
    sg = sg.reshape(144)
    cloa = cloa.reshape(144)
    bdel = inputs["w_delta_b"].reshape(144)
    wub = inputs["w_u_b"]
    bov = (inputs["w_o_b"] + inputs["e_deform"].reshape(-1))
    cv = np.zeros((112, 16), np.float32)
    cv[0:96, 0] = wub[0:96]; cv[0:96, 1] = wub[96:192]
    cv[0:96, 2] = inputs["ln_u_g"][0:96]; cv[0:96, 3] = inputs["ln_u_g"][96:192]
    cv[0:96, 4] = inputs["ln_u_b"][0:96]; cv[0:96, 5] = inputs["ln_u_b"][96:192]
    cv[0:112, 6] = bdel[0:112]; cv[0:32, 7] = bdel[112:144]
    cv[0:72, 8] = inputs["w_a_b"]
    cv[0:112, 9] = sg[0:112]; cv[0:32, 10] = sg[112:144]
    cv[0:112, 11] = cloa[0:112]; cv[0:32, 12] = cloa[112:144]
    cv[0:96, 13] = bov[0:96]; cv[0:96, 14] = bov[96:192]
    cv[0:96, 15] = 1e-5
    consts["cvec"] = cv
    consts["bd6"] = np.kron(np.eye(H, dtype=np.float32),
                            np.ones((12, 12), np.float32)).astype(ml_dtypes.bfloat16)
    io = np.zeros((128, 2 * HATB), np.float32)
    for coord in range(2):
        for l in range(NL):
            w = WXY[l]
            for hh in range(H):
                for mm in range(M):
                    st = coord * HATB + HOFF[l] + 72 * hh + w * mm
                    io[:, st:st + w] = np.arange(w, dtype=np.float32)
    consts["iotah"] = io
    consts["onesw"] = np.ones((96, 96), ml_dtypes.bfloat16)
    consts["identf"] = np.eye(128, dtype=np.float32)
    consts["identb"] = np.eye(128, dtype=ml_dtypes.bfloat16)
    woT = np.ascontiguousarray(inputs["w_o_w"].T).astype(np.float32)
    consts["wo1"] = woT[0:128].astype(ml_dtypes.bfloat16)
    consts["wo2"] = woT[128:192].astype(ml_dtypes.bfloat16)

    pmaps = []
    for b in range(B):
        pm = []
        for l in range(NL):
            Wl = maps[l].shape[3]
            mp = np.transpose(maps[l][b], (1, 2, 0))
            Hp = 32 * SCALE[l] + WXY[l]
            out = np.zeros((Hp, Hp, D), np.float32)
            out[PADL[l]:PADL[l] + Wl, PADL[l]:PADL[l] + Wl] = mp
            pm.append(out.astype(ml_dtypes.bfloat16))
        pmaps.append(pm)

    freqs = 2.0 ** np.arange(NF, dtype=np.float32)
    cell_of = ti.reshape(B, K * R)

    # rank -> (chunk, idx-in-chunk, cap)
    rank_pos = []
    r = 0
    for q, (cap, mm) in enumerate(chunks):
        for j in range(mm):
            rank_pos.append((q, j, cap))
            r += 1
    assert len(rank_pos) == 256

    in_maps, slot_maps = [], []
    for q8 in range(8):
        d = dict(consts)
        cells = deal[:, q8]  # global cell ids (b*1024+cell) by rank
        bs = cells // 1024
        cid = cells % 1024
        ayc, axc = cid // 32, cid % 32

        # patch blob: per chunk [122, m*128 ch-lo | m*64 ch-hi]
        pats = np.zeros((256, KWIN, D), ml_dtypes.bfloat16)
        for l in range(NL):
            w = WXY[l]
            ko = LOFF[l]
            for b in range(B):
                sel = np.nonzero(bs == b)[0]
                if len(sel) == 0:
                    continue
                pm = pmaps[b][l]
                ys = (SCALE[l] * ayc[sel])[:, None] + np.arange(w)
                xs = (SCALE[l] * axc[sel])[:, None] + np.arange(w)
                pt = pm[ys[:, :, None], xs[:, None, :], :]  # [n,w,w,D]
                pats[sel, ko:ko + w * w] = pt.reshape(len(sel), w * w, D)
        for qq, (cap, mm) in enumerate(chunks):
            r0 = sum(c[1] for c in chunks[:qq])
            blk = pats[r0:r0 + mm]  # [m, 122, 192]
            lo = blk[:, :, 0:128].transpose(1, 0, 2).reshape(KWIN, mm * 128)
            hi = blk[:, :, 128:192].transpose(1, 0, 2).reshape(KWIN, mm * 64)
            pbq = np.zeros((128, mm * D), ml_dtypes.bfloat16)
            pbq[:KWIN, :mm * 128] = lo
            pbq[:KWIN, mm * 128:] = hi
            d[f"pblob{qq}"] = pbq

        # slot assignment
        slot_tok = -np.ones(S, np.int64)
        slot_cell = np.zeros(S, np.int64)  # global cell id per slot
        for rank in range(256):
            qq, j, cap = rank_pos[rank]
            gcell = cells[rank]
            b = gcell // 1024
            c = gcell % 1024
            toks = np.nonzero(cell_of[b] == c)[0]
            assert len(toks) <= cap, f"rank {rank} cell {gcell} overflow {len(toks)}>{cap}"
            s0 = qq * 128 + j * cap
            slot_tok[s0:s0 + len(toks)] = b * (K * R) + toks
            slot_cell[s0:s0 + cap] = gcell
        valid = slot_tok >= 0
        st = np.where(valid, slot_tok, 0)
        b_of = st // (K * R)
        t_of = st % (K * R)
        k_of = t_of // R
        cid_of = slot_cell % 1024
        h_s = h[b_of, k_of] * valid[:, None]
        g_s = g[b_of, cid_of] * valid[:, None]
        qc_s = qc[b_of, k_of]
        ax = (cid_of % 32).astype(np.float32)
        ay = (cid_of // 32).astype(np.float32)
        anchor = np.stack([ax * 32 + 16, ay * 32 + 16], -1)
        dp = (anchor - qc_s) / 1024.0
        xf = dp[:, 0:1] * freqs * 2 * np.pi
        yf = dp[:, 1:2] * freqs * 2 * np.pi
        phi = np.concatenate([np.sin(xf), np.cos(xf), np.sin(yf), np.cos(yf)],
                             -1).astype(np.float32) * valid[:, None]
        u_in = np.concatenate([h_s, g_s, phi], -1)
        d["uinT"] = np.ascontiguousarray(u_in.T).astype(ml_dtypes.bfloat16)
        in_maps.append(d)
        slot_maps.append((slot_tok, valid))
    return in_maps, slot_maps


def kernel(**inputs):
    ti = inputs["top_indices"].astype(np.int64)
    chunks, deal, allc = _plan(ti)
    if _CACHE.get("chunks") != chunks:
        _CACHE["nc"] = _build_module(chunks)
        _CACHE["chunks"] = chunks
    nc = _CACHE["nc"]
    in_maps, slot_maps = _host_prep(inputs, chunks, deal, allc)
    res = run_bass_kernel_spmd(nc, in_maps, core_ids=list(range(8)),
                               **_CACHE.get("run_kwargs", {}))
    _CACHE["last"] = res
    B, K, R = ti.shape
    out = np.zeros((B, K * R, D), np.float32)
    for q in range(8):
        oT = np.asarray(res.results[q]["outT"], np.float32)
        slot_tok, valid = slot_maps[q]
        gt = slot_tok[valid]
        out[gt // (K * R), gt % (K * R)] = oT.T[valid]
    return out.reshape(B, K, R, D)
